# revision 1
# baseline (speedup 1.0000x reference)
"""BiLSTM-CRF NLL loss kernel for nn_BiLSTM_CRF_13889924235662 on 8 TRN2 cores.

kernel(**inputs) takes the FULL unsharded inputs (B=256, S=512) and returns
the scalar NLL (logZ - gold_score summed over batch) as float32.

Device strategy (data-parallel over batch, 32 sequences per core):
  - embedding rows gathered on-device via indirect DMA (128-row tiles),
    PE-transposed into a bf16 xT buffer with a ones-row for the biases
  - input projections (gx) computed by the TensorEngine directly into PSUM
    blocks of 8 timesteps; the LSTM recurrence accumulates W_hh·h on top and
    runs as two interleaved serial chains (fwd + bwd) per core
  - emissions GEMM -> exp(em - kappa) and raw emissions kept on-chip
  - CRF partition function in linear space: alpha from t=0 and beta from
    t=S-1 meet at S/2 (halves the serial depth), renormalized every 16 steps
  - gold-path emission score via one-hot compare + fused reduce
  - per-core scalars [sum logZ, sum em_gold] are combined on host with the
    tags-only terms (start/trans/end), which depend only on host-visible ints.

Falls back to a pure numpy implementation if the mask is not all-ones or the
device path fails for any reason.
"""

import os
import sys
import numpy as np

V, E, H, T = 100000, 100, 100, 25
PAD = T
S_FULL = 512
B_FULL = 256
BSH = 32
KAPPA = 3.25
RENORM = 64

_DEV = {"nc": None, "exec_ns": None}


# ---------------------------------------------------------------- numpy path

def _np_sigmoid(x):
    out = np.empty_like(x)
    pos = x >= 0
    out[pos] = 1.0 / (1.0 + np.exp(-x[pos]))
    ex = np.exp(x[~pos])
    out[~pos] = ex / (1.0 + ex)
    return out


def _np_logsumexp(x, axis):
    m = np.max(x, axis=axis, keepdims=True)
    return m.squeeze(axis) + np.log(np.sum(np.exp(x - m), axis=axis))


def _numpy_kernel(sentence, tags, mask, embed_table, w_ih_f, w_hh_f, b_ih_f,
                  b_hh_f, w_ih_b, w_hh_b, b_ih_b, b_hh_b, w_out, b_out,
                  start_t, end_t, trans):
    f32 = np.float32
    B, S = sentence.shape
    sent = np.asarray(sentence).astype(np.int64)
    tg = np.asarray(tags).astype(np.int64)
    msk = np.asarray(mask).astype(bool)
    tab = np.asarray(embed_table, dtype=f32).copy()
    tab[PAD] = 0.0
    x = tab[sent]
    xs = np.ascontiguousarray(np.transpose(x, (1, 0, 2)))
    xs2 = xs.reshape(S * B, E)
    b_f = (np.asarray(b_ih_f, f32) + np.asarray(b_hh_f, f32)).astype(f32)
    b_b = (np.asarray(b_ih_b, f32) + np.asarray(b_hh_b, f32)).astype(f32)
    gx_f = (xs2 @ np.asarray(w_ih_f, f32).T + b_f).reshape(S, B, 4 * H)
    gx_b = (xs2 @ np.asarray(w_ih_b, f32).T + b_b).reshape(S, B, 4 * H)

    def lstm(gx, w_hh, reverse):
        h = np.zeros((B, H), f32)
        c = np.zeros((B, H), f32)
        hs = np.empty((S, B, H), f32)
        wT = np.ascontiguousarray(np.asarray(w_hh, f32).T)
        for t in (range(S - 1, -1, -1) if reverse else range(S)):
            g = gx[t] + h @ wT
            i = _np_sigmoid(g[:, :H])
            f = _np_sigmoid(g[:, H:2 * H])
            gg = np.tanh(g[:, 2 * H:3 * H])
            o = _np_sigmoid(g[:, 3 * H:])
            c = f * c + i * gg
            h = o * np.tanh(c)
            hs[t] = h
        return hs

    hf = lstm(gx_f, w_hh_f, False)
    hb = lstm(gx_b, w_hh_b, True)
    out = np.concatenate([hf, hb], axis=-1)
    w_out = np.asarray(w_out, f32)
    b_out = np.asarray(b_out, f32)
    emissions = (out.reshape(S * B, 2 * H) @ w_out.T + b_out).reshape(S, B, T)
    start_t = np.asarray(start_t, f32)
    end_t = np.asarray(end_t, f32)
    trans = np.asarray(trans, f32)
    mask_sb = msk.T
    tags_sb = tg.T
    score = start_t + emissions[0]
    for t in range(1, S):
        z = score[:, :, None] + trans[None]
        nxt = _np_logsumexp(z, axis=1) + emissions[t]
        score = np.where(mask_sb[t][:, None], nxt, score)
    logZ = _np_logsumexp(score + end_t, axis=1)
    ar = np.arange(B)
    tags0 = np.where(mask_sb, tags_sb, 0)
    mf = mask_sb[1:].astype(f32)
    num = start_t[tags0[0]] + emissions[0, ar, tags0[0]]
    trans_s = trans[tags0[:-1], tags0[1:]]
    em_s = np.take_along_axis(emissions[1:], tags0[1:, :, None], axis=2)[..., 0]
    num = num + ((trans_s + em_s) * mf).sum(axis=0)
    seq_ends = mask_sb.astype(np.int32).sum(axis=0) - 1
    num = num + end_t[tags0[seq_ends, ar], ]
    return np.asarray((logZ - num).sum(), dtype=f32)


# ---------------------------------------------------------------- device path

def _install_ntff_hook():
    import types
    import antenv
    if "antenv.axon_hooks" in sys.modules:
        return
    mod = types.ModuleType("antenv.axon_hooks")
    _hook = [None]
    mod.set_axon_ntff_profile_hook = lambda h: _hook.__setitem__(0, h)
    mod.get_axon_ntff_profile_hook = lambda: _hook[0]
    sys.modules["antenv.axon_hooks"] = mod
    antenv.axon_hooks = mod
    try:
        from trn_agent_boot.trn_boot import _ntff_profile_via_ctypes
        mod.set_axon_ntff_profile_hook(
            _ntff_profile_via_ctypes("/opt/axon/libaxon_pjrt.so"))
    except Exception:
        pass


def _build_program(S=S_FULL):
    from contextlib import ExitStack
    import concourse.bass as bass
    import concourse.bacc as bacc
    import concourse.tile as tile
    from concourse import mybir

    f32 = mybir.dt.float32
    bf16 = mybir.dt.bfloat16
    i32 = mybir.dt.int32
    AL = mybir.AluOpType
    AF = mybir.ActivationFunctionType

    nc = bacc.Bacc()
    NT = S * BSH
    NTILE = NT // 128

    emb = nc.declare_dram_parameter("emb", [V + 1, E], bf16, isOutput=False)
    tok_idx = nc.declare_dram_parameter("tok_idx", [128, NTILE], i32, isOutput=False)
    tags_p = nc.declare_dram_parameter("tags", [1, NT], f32, isOutput=False)
    wih_p = nc.declare_dram_parameter("wih", [2, 4, 128, 128], bf16, isOutput=False)
    whh_p = nc.declare_dram_parameter("whh", [2, 4, 128, 128], bf16, isOutput=False)
    wout_p = nc.declare_dram_parameter("wout", [2, 128, 32], bf16, isOutput=False)
    bexp_p = nc.declare_dram_parameter("bexp", [32, 1], f32, isOutput=False)
    bid_p = nc.declare_dram_parameter("bid", [32, 1], f32, isOutput=False)
    etr_p = nc.declare_dram_parameter("etr", [32, 32], f32, isOutput=False)
    etrT_p = nc.declare_dram_parameter("etrT", [32, 32], f32, isOutput=False)
    estart_p = nc.declare_dram_parameter("estart", [32, 1], f32, isOutput=False)
    eend_p = nc.declare_dram_parameter("eend", [32, 1], f32, isOutput=False)
    ident_p = nc.declare_dram_parameter("ident", [128, 128], f32, isOutput=False)
    out_p = nc.declare_dram_parameter("out", [1, 8], f32, isOutput=True)

    with ExitStack() as ctx:
        tc = ctx.enter_context(tile.TileContext(nc))
        const = ctx.enter_context(tc.tile_pool(name="const", bufs=1))
        big = ctx.enter_context(tc.tile_pool(name="big", bufs=1))

        wih = {}
        whh = {}
        for d in range(2):
            for g in range(4):
                twi = const.tile([128, 128], bf16, tag=f"wih{d}{g}", name=f"wih{d}{g}")
                nc.sync.dma_start(out=twi, in_=wih_p[d, g])
                wih[(d, g)] = twi
                twh = const.tile([128, 128], bf16, tag=f"whh{d}{g}", name=f"whh{d}{g}")
                nc.sync.dma_start(out=twh, in_=whh_p[d, g])
                whh[(d, g)] = twh
        wout = []
        for hh in range(2):
            tw = const.tile([128, 32], bf16, tag=f"wout{hh}", name=f"wout{hh}")
            nc.sync.dma_start(out=tw, in_=wout_p[hh])
            wout.append(tw)
        bexp = const.tile([32, 1], f32, name="bexp")
        nc.sync.dma_start(out=bexp, in_=bexp_p[:, :])
        bid = const.tile([32, 1], f32, name="bid")
        nc.sync.dma_start(out=bid, in_=bid_p[:, :])
        etr = const.tile([32, 32], f32, name="etr")
        nc.sync.dma_start(out=etr, in_=etr_p[:, :])
        etrT = const.tile([32, 32], f32, name="etrT")
        nc.sync.dma_start(out=etrT, in_=etrT_p[:, :])
        estart = const.tile([32, 1], f32, name="estart")
        nc.sync.dma_start(out=estart, in_=estart_p[:, :])
        eend = const.tile([32, 1], f32, name="eend")
        nc.sync.dma_start(out=eend, in_=eend_p[:, :])
        ident = const.tile([128, 128], f32, name="ident")
        nc.sync.dma_start(out=ident, in_=ident_p[:, :])
        identb = const.tile([128, 128], bf16, name="identb")
        nc.vector.tensor_copy(identb, ident)
        idxt = const.tile([128, NTILE], i32, name="idxt")
        nc.sync.dma_start(out=idxt, in_=tok_idx[:, :])
        ones_col = const.tile([32, 1], f32, name="ones_col")
        nc.vector.memset(ones_col, 1.0)
        ones_row = const.tile([1, 32], f32, name="ones_row")
        nc.vector.memset(ones_row, 1.0)

        NSEG = NTILE // 2
        xseg = [big.tile([128, 256], bf16, tag=f"xs{j}", name=f"xs{j}")
                for j in range(NSEG)]
        hs = [big.tile([128, (S + 1) * BSH], bf16, tag=f"hs{c}", name=f"hs{c}")
              for c in range(2)]
        Ebuf = big.tile([25, NT], bf16, name="Ebuf")
        emT = big.tile([25, NT], bf16, name="emT")
        Zbuf = big.tile([1, 1024], f32, name="Zbuf")
        for j in range(NSEG):
            # rows 96:128 = 1.0; rows 96:100 get overwritten by the embT
            # copies, leaving row 100 = 1.0 (the bias ones-row). Rows 101:127
            # multiply zero weight rows, so their value is irrelevant.
            nc.vector.memset(xseg[j], 0.0)
            nc.vector.memset(xseg[j][96:128, :], 1.0)
        for c in range(2):
            nc.vector.memset(hs[c], 0.0)
        nc.vector.memset(Zbuf, 1.0)

        # phase 2: gx blocks + the two LSTM chains. The embedding gathers +
        # PE transposes are emitted just-in-time (block lookahead) so they
        # overlap the recurrence; the bwd gx pool is single-buffered to free
        # a PSUM bank pair for the transposes.
        cst = [const.tile([H, BSH], f32, tag=f"cst{c}", name=f"cst{c}")
               for c in range(2)]
        for c in range(2):
            nc.vector.memset(cst[c], 0.0)

        with tc.tile_pool(name="gxf", bufs=2, space="PSUM") as gxf_pool, \
             tc.tile_pool(name="gxb", bufs=1, space="PSUM") as gxb_pool, \
             tc.tile_pool(name="tp_ps", bufs=2, space="PSUM") as tp_ps, \
             tc.tile_pool(name="gat", bufs=6) as gat, \
             tc.tile_pool(name="lwork", bufs=3) as lwork:
            gx_pools = [gxf_pool, gxb_pool]
            cur_blk = [None, None]
            done_tiles = set()

            def emit_gather(k):
                if k in done_tiles or k < 0 or k >= NTILE:
                    return
                done_tiles.add(k)
                xgb = gat.tile([128, 128], bf16, tag="xgb", name="xgb")
                nc.gpsimd.indirect_dma_start(
                    out=xgb[:, 0:E], out_offset=None,
                    in_=emb[:, :],
                    in_offset=bass.IndirectOffsetOnAxis(ap=idxt[:, k:k + 1], axis=0),
                )
                pt = tp_ps.tile([128, 128], bf16, tag="pt", name="pt")
                nc.tensor.transpose(out=pt[0:E, :], in_=xgb[:, 0:E],
                                    identity=identb[:, :])
                nc.vector.tensor_copy(
                    xseg[k // 2][0:E, (k % 2) * 128:(k % 2 + 1) * 128], pt[0:E, :])

            def emit_gx_block(d, bi):
                # column layout (gate, step, b); each matmul writes one
                # contiguous 256-col gate stripe; start=True on each PSUM
                # bank's first writer (gates 0 and 2) clears the bank.
                ps = gx_pools[d].tile([128, 1024], f32, tag=f"gx{d}", name=f"gx{d}")
                seg = bi if d == 0 else (S // 8) - 1 - bi
                rhs = xseg[seg][:, :]
                for g in (0, 2, 1, 3):
                    nc.tensor.matmul(
                        ps[:, g * 256:(g + 1) * 256], wih[(d, g)], rhs,
                        start=(g in (0, 2)), stop=False, skip_group_check=True,
                    )
                return ps

            def lstm_step(d, k):
                bi, s = divmod(k, 8)
                if d == 1:
                    s = 7 - s  # bwd chain walks its block's slots backwards
                if cur_blk[d] is None or k % 8 == 0:
                    cur_blk[d] = emit_gx_block(d, bi)
                ps = cur_blk[d]
                if d == 0:
                    rd_col, wr_col = k, k + 1
                else:
                    rd_col, wr_col = S - k, S - 1 - k
                h_prev = hs[d][:, rd_col * BSH:(rd_col + 1) * BSH]
                for g in (0, 2, 1, 3):
                    nc.tensor.matmul(
                        ps[:, g * 256 + s * BSH:g * 256 + (s + 1) * BSH],
                        whh[(d, g)], h_prev,
                        start=False, stop=(g == 3), skip_group_check=True,
                    )
                gates = lwork.tile([H, 128], f32, tag=f"gates{d}", name=f"gates{d}")
                ps_step = ps[0:H, :].rearrange("p (g s b) -> p g s b", g=4, s=8)[:, :, s, :]
                nc.scalar.activation(
                    gates[:, :].rearrange("p (g b) -> p g b", g=4), ps_step, AF.Sigmoid)
                i_g = gates[:, 0:BSH]
                f_g = gates[:, BSH:2 * BSH]
                s_g = gates[:, 2 * BSH:3 * BSH]   # sigma(2g): tanh fold
                o_g = gates[:, 3 * BSH:4 * BSH]
                A2 = lwork.tile([H, BSH], f32, tag=f"A2{d}", name=f"A2{d}")
                nc.vector.scalar_tensor_tensor(A2, s_g, 2.0, i_g, AL.mult, AL.mult)
                Dt = lwork.tile([H, BSH], f32, tag=f"D{d}", name=f"D{d}")
                nc.vector.scalar_tensor_tensor(Dt, A2, 0.0, i_g, AL.add, AL.subtract)
                Bt = lwork.tile([H, BSH], f32, tag=f"B{d}", name=f"B{d}")
                nc.vector.tensor_mul(Bt, f_g, cst[d])
                nc.vector.tensor_add(cst[d], Dt, Bt)
                th = lwork.tile([H, BSH], f32, tag=f"th{d}", name=f"th{d}")
                nc.scalar.activation(th, cst[d], AF.Tanh)
                nc.vector.scalar_tensor_tensor(
                    hs[d][0:H, wr_col * BSH:(wr_col + 1) * BSH],
                    th, 0.0, o_g, AL.add, AL.mult,
                )

            # preload 4 blocks of tiles for both chains
            for pb in range(4):
                emit_gather(2 * pb)
                emit_gather(2 * pb + 1)
                emit_gather(NTILE - 2 * pb - 1)
                emit_gather(NTILE - 2 * pb - 2)
            for k in range(S):
                if k % 4 == 0:
                    nb = k // 8 + 4  # 4 blocks of lookahead, 2 tiles per call
                    if k % 8 == 0:
                        emit_gather(2 * nb)
                        emit_gather(NTILE - 2 * nb - 1)
                    else:
                        emit_gather(2 * nb + 1)
                        emit_gather(NTILE - 2 * nb - 2)
                lstm_step(0, k)
                lstm_step(1, k)

        # phase 3: emissions chunks (middle-out so the CRF can start early)
        NCH = S // 16
        with tc.tile_pool(name="em_ps", bufs=4, space="PSUM") as em_ps:
            order = []
            lo, hi = NCH // 2 - 1, NCH // 2
            while lo >= 0:
                order += [lo, hi]
                lo -= 1
                hi += 1
            for cc in order:
                ps = em_ps.tile([32, 512], f32, tag="em", name="em")
                rhs_f = hs[0][:, (16 * cc + 1) * BSH:(16 * cc + 17) * BSH]
                rhs_b = hs[1][:, (16 * cc) * BSH:(16 * cc + 16) * BSH]
                nc.tensor.matmul(ps[:, :], wout[0], rhs_f, start=True, stop=False)
                nc.tensor.matmul(ps[:, :], wout[1], rhs_b, start=False, stop=True)
                cols = slice(cc * 512, (cc + 1) * 512)
                nc.scalar.activation(Ebuf[0:T, cols], ps[0:T, :], AF.Exp,
                                     bias=bexp[0:T, :])
                nc.scalar.activation(emT[0:T, cols], ps[0:T, :], AF.Identity,
                                     bias=bid[0:T, :])

        # phase 4: gold score + the two CRF half-chains
        half = S // 2
        with tc.tile_pool(name="crf", bufs=2) as crf, \
             tc.tile_pool(name="crf_ps", bufs=2, space="PSUM") as crf_ps, \
             tc.tile_pool(name="gold", bufs=2) as gold, \
             tc.tile_pool(name="fin", bufs=1) as fin:

            iof = fin.tile([32, 1], f32, name="iof")
            io32 = fin.tile([32, 1], i32, name="io32")
            nc.gpsimd.iota(io32, pattern=[[0, 1]], base=0, channel_multiplier=1)
            nc.vector.tensor_copy(iof, io32)
            ga_tot = fin.tile([32, 1], f32, name="ga_tot")
            nc.vector.memset(ga_tot, 0.0)
            GW = min(1024, NT)
            for cc in range(NT // GW):
                tgc = gold.tile([1, GW], f32, tag="tgc", name="tgc")
                nc.sync.dma_start(out=tgc, in_=tags_p[:, cc * GW:(cc + 1) * GW])
                tb = gold.tile([T, GW], f32, tag="tb", name="tb")
                nc.gpsimd.partition_broadcast(tb[:, :], tgc[:, :])
                sc = gold.tile([T, GW], bf16, tag="sc", name="sc")
                ga = gold.tile([T, 1], f32, tag="ga", name="ga")
                nc.vector.scalar_tensor_tensor(
                    sc, tb, iof[0:T, :], emT[0:T, cc * GW:(cc + 1) * GW],
                    AL.is_equal, AL.mult, accum_out=ga,
                )
                nc.vector.tensor_add(ga_tot[0:T, :], ga_tot[0:T, :], ga)

            al = crf.tile([T, BSH], f32, tag="al", name="al")
            be = crf.tile([T, BSH], f32, tag="be", name="be")
            nc.vector.tensor_scalar(al, Ebuf[0:T, 0:BSH], estart[0:T, :], None, AL.mult)
            nc.vector.memset(be, 1.0)
            nc.vector.tensor_scalar(be, be, eend[0:T, :], None, AL.mult)

            state = {"al": al, "be": be}
            nren = [0, 0]

            def crf_step(which, t, do_renorm):
                st = state[which]
                Esl = Ebuf[0:T, t * BSH:(t + 1) * BSH]
                q = crf_ps.tile([32, BSH], f32, tag="q", name=f"q{which}")
                if which == "al":
                    nc.tensor.matmul(q[0:T, :], etr[0:T, 0:T], st, start=True, stop=True)
                    nxt = crf.tile([T, BSH], f32, tag="al", name="al_n")
                    nc.vector.scalar_tensor_tensor(nxt, q[0:T, :], 0.0, Esl,
                                                   AL.add, AL.mult)
                else:
                    v = crf.tile([T, BSH], f32, tag="bev", name="bev")
                    nc.vector.scalar_tensor_tensor(v, st, 0.0, Esl, AL.add, AL.mult)
                    nc.tensor.matmul(q[0:T, :], etrT[0:T, 0:T], v, start=True, stop=True)
                    nxt = crf.tile([T, BSH], f32, tag="be", name="be_n")
                    nc.vector.tensor_copy(nxt, q[0:T, :])
                if do_renorm:
                    zp = crf_ps.tile([32, BSH], f32, tag="zr", name=f"zp{which}")
                    nc.tensor.matmul(zp[0:1, :], ones_col[0:T, :], nxt,
                                     start=True, stop=True)
                    idx = 0 if which == "al" else 1
                    slot = nren[idx] + (0 if which == "al" else 16)
                    nren[idx] += 1
                    zcopy = Zbuf[0:1, :].rearrange("p (b r) -> p b r", r=32)[:, :, slot]
                    nc.vector.tensor_copy(zcopy, zp[0:1, :])
                    zi = crf.tile([1, BSH], f32, tag=f"zi{which}", name=f"zi{which}")
                    nc.vector.reciprocal(zi, zp[0:1, :])
                    zb = crf_ps.tile([32, BSH], f32, tag="zr", name=f"zb{which}")
                    nc.tensor.matmul(zb[0:T, :], ones_row[:, 0:T], zi,
                                     start=True, stop=True)
                    nrm = crf.tile([T, BSH], f32,
                                   tag="al" if which == "al" else "be", name="nrm")
                    nc.vector.scalar_tensor_tensor(nrm, zb[0:T, :], 0.0, nxt,
                                                   AL.add, AL.mult)
                    nxt = nrm
                state[which] = nxt

            for k in range(1, half):
                crf_step("al", k, k % RENORM == 0)
                crf_step("be", S - k, k % RENORM == 0)
            crf_step("be", half, False)

            sfin = fin.tile([T, BSH], f32, name="sfin")
            nc.vector.tensor_mul(sfin, state["al"], state["be"])
            zfin = crf_ps.tile([32, BSH], f32, tag="zr", name="zfin")
            nc.tensor.matmul(zfin[0:1, :], ones_col[0:T, :], sfin, start=True, stop=True)
            logf = fin.tile([1, BSH], f32, name="logf")
            nc.scalar.activation(logf, zfin[0:1, :], AF.Ln)
            zlog = fin.tile([1, 1024], f32, name="zlog")
            nc.scalar.activation(zlog, Zbuf[:, :], AF.Ln)
            rsum = fin.tile([1, BSH], f32, name="rsum")
            nc.vector.tensor_reduce(
                rsum[0:1, :].rearrange("p (b o) -> p b o", o=1),
                zlog[0:1, :].rearrange("p (b r) -> p b r", r=32),
                axis=mybir.AxisListType.X, op=AL.add,
            )
            lz = fin.tile([1, BSH], f32, name="lz")
            nc.vector.tensor_add(lz, logf, rsum)
            nc.vector.tensor_scalar_add(lz, lz, float(S * KAPPA))
            lzt = fin.tile([1, 1], f32, name="lzt")
            nc.vector.tensor_reduce(lzt, lz, axis=mybir.AxisListType.X, op=AL.add)
            gfin = crf_ps.tile([32, 1], f32, tag="zr", name="gfin")
            nc.tensor.matmul(gfin[0:1, :], ones_col[0:T, :], ga_tot[0:T, :],
                             start=True, stop=True)
            ot = fin.tile([1, 8], f32, name="ot")
            nc.vector.memset(ot, 0.0)
            nc.vector.tensor_copy(ot[0:1, 0:1], lzt)
            nc.vector.tensor_copy(ot[0:1, 1:2], gfin[0:1, :])
            nc.sync.dma_start(out=out_p[:, :], in_=ot)

    nc.compile()
    return nc


def _pack_inputs(inputs, S=S_FULL):
    import ml_dtypes
    f = np.float32
    sent = np.asarray(inputs["sentence"]).astype(np.int64)
    tags = np.asarray(inputs["tags"]).astype(np.int64)
    import ml_dtypes as _mld
    emb = np.asarray(inputs["embed_table"], dtype=f).copy()
    emb[PAD] = 0.0
    emb = emb.astype(_mld.bfloat16)
    B = sent.shape[0]
    ncores = B // BSH

    def bf(x):
        return np.asarray(x, dtype=f).astype(ml_dtypes.bfloat16)

    wih = np.zeros((2, 4, 128, 128), f)
    whh = np.zeros((2, 4, 128, 128), f)
    for d, (wi, wh, bi, bh) in enumerate([
        (inputs["w_ih_f"], inputs["w_hh_f"], inputs["b_ih_f"], inputs["b_hh_f"]),
        (inputs["w_ih_b"], inputs["w_hh_b"], inputs["b_ih_b"], inputs["b_hh_b"]),
    ]):
        wi = np.asarray(wi, dtype=f)
        wh = np.asarray(wh, dtype=f)
        bb = np.asarray(bi, dtype=f) + np.asarray(bh, dtype=f)
        for g in range(4):
            scale = 2.0 if g == 2 else 1.0
            wih[d, g, 0:E, 0:H] = wi[g * H:(g + 1) * H, :].T * scale
            wih[d, g, E, 0:H] = bb[g * H:(g + 1) * H] * scale
            whh[d, g, 0:H, 0:H] = wh[g * H:(g + 1) * H, :].T * scale
    wo = np.asarray(inputs["w_out"], dtype=f)
    wout = np.zeros((2, 128, 32), f)
    wout[0, 0:H, 0:T] = wo[:, 0:H].T
    wout[1, 0:H, 0:T] = wo[:, H:2 * H].T

    bexp = np.zeros((32, 1), f)
    bexp[0:T, 0] = np.asarray(inputs["b_out"], dtype=f) - KAPPA
    bid = np.zeros((32, 1), f)
    bid[0:T, 0] = np.asarray(inputs["b_out"], dtype=f)
    etr = np.zeros((32, 32), f)
    etr[0:T, 0:T] = np.exp(np.asarray(inputs["trans"], dtype=f))
    etrT = np.zeros((32, 32), f)
    etrT[0:T, 0:T] = etr[0:T, 0:T].T
    estart = np.zeros((32, 1), f)
    estart[0:T, 0] = np.exp(np.asarray(inputs["start_t"], dtype=f))
    eend = np.zeros((32, 1), f)
    eend[0:T, 0] = np.exp(np.asarray(inputs["end_t"], dtype=f))
    ident = np.eye(128, dtype=f)
    wih_b, whh_b, wout_b = bf(wih), bf(whh), bf(wout)

    in_maps = []
    for c in range(ncores):
        bs = slice(c * BSH, (c + 1) * BSH)
        tok = sent[bs, :S].T.reshape(-1).astype(np.int32)
        tgf = tags[bs, :S].T.reshape(1, -1).astype(f)
        NT = S * BSH
        idx = tok.reshape(NT // 128, 128).T.copy()
        in_maps.append(dict(
            emb=emb, tok_idx=idx, tags=tgf,
            wih=wih_b, whh=whh_b, wout=wout_b,
            bexp=bexp, bid=bid, etr=etr, etrT=etrT,
            estart=estart, eend=eend, ident=ident,
        ))
    return in_maps


def _host_terms(inputs):
    f = np.float32
    tags = np.asarray(inputs["tags"]).astype(np.int64)
    start_t = np.asarray(inputs["start_t"], dtype=f)
    end_t = np.asarray(inputs["end_t"], dtype=f)
    trans = np.asarray(inputs["trans"], dtype=f)
    tg = tags.T
    return float(start_t[tg[0]].sum() + trans[tg[:-1], tg[1:]].sum()
                 + end_t[tg[-1]].sum())


def _device_kernel(inputs):
    from concourse.bass_utils import run_bass_kernel_spmd
    _install_ntff_hook()
    if _DEV["nc"] is None:
        _DEV["nc"] = _build_program(S=S_FULL)
    in_maps = _pack_inputs(inputs, S=S_FULL)
    trace = os.environ.get("KERNEL_TRACE", "0") == "1"
    kwargs = {}
    if trace:
        import tempfile
        kwargs["tmpdir"] = tempfile.mkdtemp(prefix="kernel_trace_")
    res = run_bass_kernel_spmd(_DEV["nc"], in_maps, list(range(8)),
                               trace=trace, **kwargs)
    _DEV["exec_ns"] = res.exec_time_ns
    _DEV["profile_json"] = getattr(res, "profile_json", None)
    tot_logZ = sum(float(res.results[c]["out"][0, 0]) for c in range(8))
    tot_gold = sum(float(res.results[c]["out"][0, 1]) for c in range(8))
    loss = tot_logZ - tot_gold - _host_terms(inputs)
    return np.asarray(loss, dtype=np.float32)


def kernel(sentence, tags, mask, embed_table, w_ih_f, w_hh_f, b_ih_f, b_hh_f,
           w_ih_b, w_hh_b, b_ih_b, b_hh_b, w_out, b_out, start_t, end_t, trans):
    inputs = dict(
        sentence=sentence, tags=tags, mask=mask, embed_table=embed_table,
        w_ih_f=w_ih_f, w_hh_f=w_hh_f, b_ih_f=b_ih_f, b_hh_f=b_hh_f,
        w_ih_b=w_ih_b, w_hh_b=w_hh_b, b_ih_b=b_ih_b, b_hh_b=b_hh_b,
        w_out=w_out, b_out=b_out, start_t=start_t, end_t=end_t, trans=trans,
    )
    msk = np.asarray(mask)
    shapes_ok = (np.asarray(sentence).shape == (B_FULL, S_FULL))
    if shapes_ok and bool(msk.all()) and os.environ.get("KERNEL_FORCE_NUMPY") != "1":
        try:
            return _device_kernel(inputs)
        except Exception as e:  # fall back to the host path on any failure
            print(f"kernel: device path failed ({type(e).__name__}: {e}); "
                  f"falling back to numpy", file=sys.stderr)
    return _numpy_kernel(**inputs)

